# revision 20
# baseline (speedup 1.0000x reference)
"""Joint soft-histogram kernel for Trainium2 (Bass/Tile), 8-core data parallel.

Math (per batch b, K=256, L=1/256, W=L/2.5, N=65536 pixels):
    phi_k(x) = S_k(x) - S_{k+1}(x),   S_k(x) = sigmoid(640*x - 2.5*k)
    out[k, j] = sum_n phi_k(x_n) * phi_j(y_n) / N

Double telescope: out = Drow(Dcol(M)) / N with M = Sx^T @ Sy (257 x 257),
M[k, j] = sum_n S_k(x_n) * S_j(y_n). Neither side needs a per-chunk adjacent
difference -- both collapse onto the tiny M. M entries grow to O(N), so PSUM
fp32 accumulation is drained to SBUF every SEG chunks (caps entries at
SEG*128 = 8192, keeping roundoff ~1e-3 absolute, ~4e-3 relative after
differencing -- inside the 2e-2 budget).

Engine plan (v3; v1 spent 484us in per-chunk DVE TENSOR_SCALARs + 338us in
GPSIMD diffs; v2's PE rank-2 preadds measured 2.4x slower than modeled):
  - preadd A[p, c*KP+j] = 640*v[p,c] - 2.5*j: ONE broadcast-AP tensor_tensor
    per 16-chunk group (measured 4.4us/group), writing SBUF.
  - sigmoid: one big staged ACTIVATE per group (measured 3.7us/group). This
    ~238us of ACT work is the dense-algorithm floor.
  - a tunable number of preadd groups go to GPSIMD to unload DVE.
  - PE: 2x 128-row matmuls + 1-row tail matmul per chunk, fp16, plus segment
    restarts (start=True zeroes PSUM).

Sharding: pure data parallel, batch b -> core b.
"""

import numpy as np

import concourse.bass as bass
import concourse.tile as tile
from concourse import bacc, mybir
from concourse.bass_utils import run_bass_kernel_spmd

F32 = mybir.dt.float32
F16 = mybir.dt.float16

B = 8
K = 256
KB = K + 1            # 257 sigmoid taps per side (k = 0..256)
KP = K + 2            # 258: per-chunk stride in staged tiles (even)
NPIX = 65536
NCHUNK = 512
XG = 16               # chunks per staged group
NG = NCHUNK // XG     # 32 groups
GF = XG * KP          # staged group free size (4128)
INV_N = 1.0 / NPIX
SEG = 64              # chunks per PSUM accumulation segment
NSEG = NCHUNK // SEG

# --- tuning knobs -----------------------------------------------------------
# Each preadd group's broadcast-TT is split: chunks [0:VSPLIT] on DVE,
# [VSPLIT:XG] on GPSIMD (concurrent).  VSPLIT=XG disables the GPSIMD share.
VSPLIT = 12
# ---------------------------------------------------------------------------

_cached_nc = None


def _build():
    nc = bacc.Bacc("TRN2")
    xd = nc.declare_dram_parameter("x", [128, 512], F32, isOutput=False)
    yd = nc.declare_dram_parameter("y", [128, 512], F32, isOutput=False)
    kd = nc.declare_dram_parameter("krow", [128, KP], F32, isOutput=False)
    # dmat[k, k'] = [k==k'] - [k==k'+1]; dnext[k, k'] = -[k==0][k'==127]
    dmd = nc.declare_dram_parameter("dmat", [128, 128], F32, isOutput=False)
    dnd = nc.declare_dram_parameter("dnext", [128, 128], F32, isOutput=False)
    od = nc.declare_dram_parameter("out", [256, 256], F32, isOutput=True)

    sig = mybir.ActivationFunctionType.Sigmoid
    add = mybir.AluOpType.add

    with tile.TileContext(nc) as tc:
        with (
            tc.tile_pool(name="singles", bufs=1) as singles,
            tc.tile_pool(name="stage32", bufs=3) as stage32,
            tc.tile_pool(name="stage16", bufs=3) as stage16,
            tc.tile_pool(name="work", bufs=4) as work,
            tc.tile_pool(name="psum", bufs=1, space="PSUM") as psum,
        ):
            xt = singles.tile([128, 512], F32)
            nc.sync.dma_start(out=xt, in_=xd[:, :])
            yt = singles.tile([128, 512], F32)
            nc.sync.dma_start(out=yt, in_=yd[:, :])
            kr = singles.tile([128, KP], F32)
            nc.sync.dma_start(out=kr, in_=kd[:, :])
            dm = singles.tile([128, 128], F32)
            nc.sync.dma_start(out=dm, in_=dmd[:, :])
            dn = singles.tile([128, 128], F32)
            nc.sync.dma_start(out=dn, in_=dnd[:, :])

            # M accumulators in SBUF: rows 0..127 / 128..255 / 256 (tail)
            acc = singles.tile([128, 2, KB], F32)
            acct = singles.tile([128, KB], F32)  # only partition 0 used
            nc.vector.memset(acc, 0.0)
            nc.vector.memset(acct[0:1, :], 0.0)

            # PSUM: M' segment accumulator (rows 0..255 + tail row 256)
            Mp = psum.tile([128, 2, 512], F32)
            Mt = psum.tile([128, 512], F32)  # partition 0 = row 256

            def preadd_sigmoid(src, g, tag):
                a = stage32.tile([128, XG, KP], F32, tag="a" + tag)
                c0 = g * XG
                nc.vector.tensor_tensor(
                    out=a[:, 0:VSPLIT, :],
                    in0=src[:, c0:c0 + VSPLIT].unsqueeze(2)
                        .broadcast_to([128, VSPLIT, KP]),
                    in1=kr.unsqueeze(1).broadcast_to([128, VSPLIT, KP]),
                    op=add,
                )
                if VSPLIT < XG:
                    nc.gpsimd.tensor_tensor(
                        out=a[:, VSPLIT:XG, :],
                        in0=src[:, c0 + VSPLIT:c0 + XG].unsqueeze(2)
                            .broadcast_to([128, XG - VSPLIT, KP]),
                        in1=kr.unsqueeze(1)
                            .broadcast_to([128, XG - VSPLIT, KP]),
                        op=add,
                    )
                s = stage16.tile([128, XG, KP], F16, tag="s" + tag)
                nc.scalar.activation(out=s, in_=a, func=sig)
                return s

            for g in range(NG):
                sx = preadd_sigmoid(xt, g, "x")
                sy = preadd_sigmoid(yt, g, "y")
                for i in range(XG):
                    c = g * XG + i
                    first = c % SEG == 0
                    last = c % SEG == SEG - 1
                    ty = sy[:, i, 0:KB]
                    nc.tensor.matmul(
                        Mp[:, 0, 0:KB],
                        lhsT=sx[:, i, 0:128],
                        rhs=ty,
                        start=first,
                        stop=last,
                    )
                    nc.tensor.matmul(
                        Mp[:, 1, 0:KB],
                        lhsT=sx[:, i, 128:256],
                        rhs=ty,
                        start=first,
                        stop=last,
                    )
                    nc.tensor.matmul(
                        Mt[0:1, 0:KB],
                        lhsT=sx[:, i, 256:257],
                        rhs=ty,
                        start=first,
                        stop=last,
                    )
                    if last:
                        # drain segment into SBUF accumulators
                        for h in range(2):
                            nc.vector.tensor_add(
                                out=acc[:, h, :], in0=acc[:, h, :],
                                in1=Mp[:, h, 0:KB],
                            )
                        nc.vector.tensor_add(
                            out=acct[0:1, :], in0=acct[0:1, :],
                            in1=Mt[0:1, 0:KB],
                        )

            # Epilogue: out[k, j] = (Mr[k, j] - Mr[k, j+1]) / N with
            # Mr[k, j] = acc[k, j] - acc[k+1, j], row diff via PE:
            # rd_h = dmat^T @ acc_h + dnext^T @ acc_{h+1}.
            for h in range(2):
                rd = psum.tile([128, 512], F32, tag="rd")
                nc.tensor.matmul(
                    rd[:, 0:KB], lhsT=dm, rhs=acc[:, h, :],
                    start=True, stop=False,
                )
                nxt = acc[:, 1, :] if h == 0 else acct[:, :]
                nc.tensor.matmul(
                    rd[:, 0:KB], lhsT=dn, rhs=nxt,
                    start=False, stop=True,
                )
                t1 = work.tile([128, KB], F32, tag="ep")
                nc.scalar.activation(
                    out=t1, in_=rd[:, 0:KB],
                    func=mybir.ActivationFunctionType.Copy, scale=INV_N,
                )
                t2 = work.tile([128, K], F32, tag="ep2")
                nc.vector.tensor_sub(out=t2, in0=t1[:, 0:K], in1=t1[:, 1:KB])
                nc.sync.dma_start(out=od[128 * h: 128 * (h + 1), :], in_=t2)

    nc.finalize()
    return nc


def _get_nc():
    global _cached_nc
    if _cached_nc is None:
        _cached_nc = _build()
    return _cached_nc


def _krow():
    row = np.arange(KP, dtype=np.float32) * np.float32(-2.5)
    return np.tile(row[None, :], (128, 1))


def _dmat():
    d = np.eye(128, dtype=np.float32)
    d -= np.eye(128, k=-1, dtype=np.float32)
    return d


def _dnext():
    d = np.zeros((128, 128), dtype=np.float32)
    d[0, 127] = -1.0
    return d


def _in_maps(x, y):
    x = np.asarray(x, dtype=np.float32)
    y = np.asarray(y, dtype=np.float32)
    kr = _krow()
    maps = []
    for b in range(B):
        x6 = np.ascontiguousarray(x[b].reshape(128, 512) * np.float32(640.0))
        y6 = np.ascontiguousarray(y[b].reshape(128, 512) * np.float32(640.0))
        maps.append({"x": x6, "y": y6, "krow": kr,
                     "dmat": _dmat(), "dnext": _dnext()})
    return maps


def run(x, y, trace=False, **trace_kw):
    """Run on all 8 cores; returns (out (8,256,256) f32, BassKernelResults)."""
    nc = _get_nc()
    res = run_bass_kernel_spmd(nc, _in_maps(x, y), list(range(B)), trace=trace,
                               **trace_kw)
    out = np.stack([res.results[b]["out"] for b in range(B)]).astype(np.float32)
    return out, res


def kernel(x, y):
    out, _ = run(x, y)
    return out


# revision 21
# speedup vs baseline: 1.0001x; 1.0001x over previous
"""Joint soft-histogram kernel for Trainium2 (Bass/Tile), 8-core data parallel.

Math (per batch b, K=256, L=1/256, W=L/2.5, N=65536 pixels):
    phi_k(x) = S_k(x) - S_{k+1}(x),   S_k(x) = sigmoid(640*x - 2.5*k)
    out[k, j] = sum_n phi_k(x_n) * phi_j(y_n) / N

Double telescope: out = Drow(Dcol(M)) / N with M = Sx^T @ Sy (257 x 257),
M[k, j] = sum_n S_k(x_n) * S_j(y_n). Neither side needs a per-chunk adjacent
difference -- both collapse onto the tiny M. M entries grow to O(N), so PSUM
fp32 accumulation is drained to SBUF every SEG chunks (caps entries at
SEG*128 = 8192, keeping roundoff ~1e-3 absolute, ~4e-3 relative after
differencing -- inside the 2e-2 budget).

Engine plan (v3; v1 spent 484us in per-chunk DVE TENSOR_SCALARs + 338us in
GPSIMD diffs; v2's PE rank-2 preadds measured 2.4x slower than modeled):
  - preadd A[p, c*KP+j] = 640*v[p,c] - 2.5*j: ONE broadcast-AP tensor_tensor
    per 16-chunk group (measured 4.4us/group), writing SBUF.
  - sigmoid: one big staged ACTIVATE per group (measured 3.7us/group). This
    ~238us of ACT work is the dense-algorithm floor.
  - a tunable number of preadd groups go to GPSIMD to unload DVE.
  - PE: 2x 128-row matmuls + 1-row tail matmul per chunk, fp16, plus segment
    restarts (start=True zeroes PSUM).

Sharding: pure data parallel, batch b -> core b.
"""

import numpy as np

import concourse.bass as bass
import concourse.tile as tile
from concourse import bacc, mybir
from concourse.bass_utils import run_bass_kernel_spmd

F32 = mybir.dt.float32
F16 = mybir.dt.float16

B = 8
K = 256
KB = K + 1            # 257 sigmoid taps per side (k = 0..256)
KP = K + 2            # 258: per-chunk stride in staged tiles (even)
NPIX = 65536
NCHUNK = 512
XG = 16               # chunks per staged group
NG = NCHUNK // XG     # 32 groups
GF = XG * KP          # staged group free size (4128)
INV_N = 1.0 / NPIX
SEG = 64              # chunks per PSUM accumulation segment
NSEG = NCHUNK // SEG

# --- tuning knobs -----------------------------------------------------------
# Each preadd group's broadcast-TT is split: chunks [0:VSPLIT] on DVE,
# [VSPLIT:XG] on GPSIMD (concurrent).  VSPLIT=XG disables the GPSIMD share.
VSPLIT = 14
# ---------------------------------------------------------------------------

_cached_nc = None


def _build():
    nc = bacc.Bacc("TRN2")
    xd = nc.declare_dram_parameter("x", [128, 512], F32, isOutput=False)
    yd = nc.declare_dram_parameter("y", [128, 512], F32, isOutput=False)
    kd = nc.declare_dram_parameter("krow", [128, KP], F32, isOutput=False)
    # dmat[k, k'] = [k==k'] - [k==k'+1]; dnext[k, k'] = -[k==0][k'==127]
    dmd = nc.declare_dram_parameter("dmat", [128, 128], F32, isOutput=False)
    dnd = nc.declare_dram_parameter("dnext", [128, 128], F32, isOutput=False)
    od = nc.declare_dram_parameter("out", [256, 256], F32, isOutput=True)

    sig = mybir.ActivationFunctionType.Sigmoid
    add = mybir.AluOpType.add

    with tile.TileContext(nc) as tc:
        with (
            tc.tile_pool(name="singles", bufs=1) as singles,
            tc.tile_pool(name="stage32", bufs=3) as stage32,
            tc.tile_pool(name="stage16", bufs=3) as stage16,
            tc.tile_pool(name="work", bufs=4) as work,
            tc.tile_pool(name="psum", bufs=1, space="PSUM") as psum,
        ):
            xt = singles.tile([128, 512], F32)
            nc.sync.dma_start(out=xt, in_=xd[:, :])
            yt = singles.tile([128, 512], F32)
            nc.sync.dma_start(out=yt, in_=yd[:, :])
            kr = singles.tile([128, KP], F32)
            nc.sync.dma_start(out=kr, in_=kd[:, :])
            dm = singles.tile([128, 128], F32)
            nc.sync.dma_start(out=dm, in_=dmd[:, :])
            dn = singles.tile([128, 128], F32)
            nc.sync.dma_start(out=dn, in_=dnd[:, :])

            # M accumulators in SBUF: rows 0..127 / 128..255 / 256 (tail)
            acc = singles.tile([128, 2, KB], F32)
            acct = singles.tile([128, KB], F32)  # only partition 0 used
            nc.vector.memset(acc, 0.0)
            nc.vector.memset(acct[0:1, :], 0.0)

            # PSUM: M' segment accumulator (rows 0..255 + tail row 256)
            Mp = psum.tile([128, 2, 512], F32)
            Mt = psum.tile([128, 512], F32)  # partition 0 = row 256

            def preadd_sigmoid(src, g, tag):
                a = stage32.tile([128, XG, KP], F32, tag="a" + tag)
                c0 = g * XG
                nc.vector.tensor_tensor(
                    out=a[:, 0:VSPLIT, :],
                    in0=src[:, c0:c0 + VSPLIT].unsqueeze(2)
                        .broadcast_to([128, VSPLIT, KP]),
                    in1=kr.unsqueeze(1).broadcast_to([128, VSPLIT, KP]),
                    op=add,
                )
                if VSPLIT < XG:
                    nc.gpsimd.tensor_tensor(
                        out=a[:, VSPLIT:XG, :],
                        in0=src[:, c0 + VSPLIT:c0 + XG].unsqueeze(2)
                            .broadcast_to([128, XG - VSPLIT, KP]),
                        in1=kr.unsqueeze(1)
                            .broadcast_to([128, XG - VSPLIT, KP]),
                        op=add,
                    )
                s = stage16.tile([128, XG, KP], F16, tag="s" + tag)
                nc.scalar.activation(out=s, in_=a, func=sig)
                return s

            for g in range(NG):
                sx = preadd_sigmoid(xt, g, "x")
                sy = preadd_sigmoid(yt, g, "y")
                for i in range(XG):
                    c = g * XG + i
                    first = c % SEG == 0
                    last = c % SEG == SEG - 1
                    ty = sy[:, i, 0:KB]
                    nc.tensor.matmul(
                        Mp[:, 0, 0:KB],
                        lhsT=sx[:, i, 0:128],
                        rhs=ty,
                        start=first,
                        stop=last,
                    )
                    nc.tensor.matmul(
                        Mp[:, 1, 0:KB],
                        lhsT=sx[:, i, 128:256],
                        rhs=ty,
                        start=first,
                        stop=last,
                    )
                    nc.tensor.matmul(
                        Mt[0:1, 0:KB],
                        lhsT=sx[:, i, 256:257],
                        rhs=ty,
                        start=first,
                        stop=last,
                    )
                    if last:
                        # drain segment into SBUF accumulators
                        for h in range(2):
                            nc.vector.tensor_add(
                                out=acc[:, h, :], in0=acc[:, h, :],
                                in1=Mp[:, h, 0:KB],
                            )
                        nc.vector.tensor_add(
                            out=acct[0:1, :], in0=acct[0:1, :],
                            in1=Mt[0:1, 0:KB],
                        )

            # Epilogue: out[k, j] = (Mr[k, j] - Mr[k, j+1]) / N with
            # Mr[k, j] = acc[k, j] - acc[k+1, j], row diff via PE:
            # rd_h = dmat^T @ acc_h + dnext^T @ acc_{h+1}.
            for h in range(2):
                rd = psum.tile([128, 512], F32, tag="rd")
                nc.tensor.matmul(
                    rd[:, 0:KB], lhsT=dm, rhs=acc[:, h, :],
                    start=True, stop=False,
                )
                nxt = acc[:, 1, :] if h == 0 else acct[:, :]
                nc.tensor.matmul(
                    rd[:, 0:KB], lhsT=dn, rhs=nxt,
                    start=False, stop=True,
                )
                t1 = work.tile([128, KB], F32, tag="ep")
                nc.scalar.activation(
                    out=t1, in_=rd[:, 0:KB],
                    func=mybir.ActivationFunctionType.Copy, scale=INV_N,
                )
                t2 = work.tile([128, K], F32, tag="ep2")
                nc.vector.tensor_sub(out=t2, in0=t1[:, 0:K], in1=t1[:, 1:KB])
                nc.sync.dma_start(out=od[128 * h: 128 * (h + 1), :], in_=t2)

    nc.finalize()
    return nc


def _get_nc():
    global _cached_nc
    if _cached_nc is None:
        _cached_nc = _build()
    return _cached_nc


def _krow():
    row = np.arange(KP, dtype=np.float32) * np.float32(-2.5)
    return np.tile(row[None, :], (128, 1))


def _dmat():
    d = np.eye(128, dtype=np.float32)
    d -= np.eye(128, k=-1, dtype=np.float32)
    return d


def _dnext():
    d = np.zeros((128, 128), dtype=np.float32)
    d[0, 127] = -1.0
    return d


def _in_maps(x, y):
    x = np.asarray(x, dtype=np.float32)
    y = np.asarray(y, dtype=np.float32)
    kr = _krow()
    maps = []
    for b in range(B):
        x6 = np.ascontiguousarray(x[b].reshape(128, 512) * np.float32(640.0))
        y6 = np.ascontiguousarray(y[b].reshape(128, 512) * np.float32(640.0))
        maps.append({"x": x6, "y": y6, "krow": kr,
                     "dmat": _dmat(), "dnext": _dnext()})
    return maps


def run(x, y, trace=False, **trace_kw):
    """Run on all 8 cores; returns (out (8,256,256) f32, BassKernelResults)."""
    nc = _get_nc()
    res = run_bass_kernel_spmd(nc, _in_maps(x, y), list(range(B)), trace=trace,
                               **trace_kw)
    out = np.stack([res.results[b]["out"] for b in range(B)]).astype(np.float32)
    return out, res


def kernel(x, y):
    out, _ = run(x, y)
    return out


# revision 22
# speedup vs baseline: 1.0719x; 1.0717x over previous
"""Joint soft-histogram kernel for Trainium2 (Bass/Tile), 8-core data parallel.

Math (per batch b, K=256, L=1/256, W=L/2.5, N=65536 pixels):
    phi_k(x) = S_k(x) - S_{k+1}(x),   S_k(x) = sigmoid(640*x - 2.5*k)
    out[k, j] = sum_n phi_k(x_n) * phi_j(y_n) / N

Double telescope: out = Drow(Dcol(M)) / N with M = Sx^T @ Sy (257 x 257),
M[k, j] = sum_n S_k(x_n) * S_j(y_n). Neither side needs a per-chunk adjacent
difference -- both collapse onto the tiny M. M entries grow to O(N), so PSUM
fp32 accumulation is drained to SBUF every SEG chunks (caps entries at
SEG*128 = 8192, keeping roundoff ~1e-3 absolute, ~4e-3 relative after
differencing -- inside the 2e-2 budget).

Engine plan (v3; v1 spent 484us in per-chunk DVE TENSOR_SCALARs + 338us in
GPSIMD diffs; v2's PE rank-2 preadds measured 2.4x slower than modeled):
  - preadd A[p, c*KP+j] = 640*v[p,c] - 2.5*j: ONE broadcast-AP tensor_tensor
    per 16-chunk group (measured 4.4us/group), writing SBUF.
  - sigmoid: one big staged ACTIVATE per group (measured 3.7us/group). This
    ~238us of ACT work is the dense-algorithm floor.
  - a tunable number of preadd groups go to GPSIMD to unload DVE.
  - PE: 2x 128-row matmuls + 1-row tail matmul per chunk, fp16, plus segment
    restarts (start=True zeroes PSUM).

Sharding: pure data parallel, batch b -> core b.
"""

import numpy as np

import concourse.bass as bass
import concourse.tile as tile
from concourse import bacc, mybir
from concourse.bass_utils import run_bass_kernel_spmd

F32 = mybir.dt.float32
F16 = mybir.dt.float16

B = 8
K = 256
KB = K + 1            # 257 sigmoid taps per side (k = 0..256)
KP = K + 2            # 258: per-chunk stride in staged tiles (even)
NPIX = 65536
NCHUNK = 512
XG = 16               # chunks per staged group
NG = NCHUNK // XG     # 32 groups
GF = XG * KP          # staged group free size (4128)
INV_N = 1.0 / NPIX
SEG = 64              # chunks per PSUM accumulation segment
NSEG = NCHUNK // SEG

# --- tuning knobs -----------------------------------------------------------
# Preadd engine per (group, side): 'v' = DVE broadcast-TT, 'g' = GPSIMD TT.
X_ENG = ['v'] * NG
Y_ENG = ['g' if g % 5 == 2 else 'v' for g in range(NG)]
# ---------------------------------------------------------------------------

_cached_nc = None


def _build():
    nc = bacc.Bacc("TRN2")
    xd = nc.declare_dram_parameter("x", [128, 512], F32, isOutput=False)
    yd = nc.declare_dram_parameter("y", [128, 512], F32, isOutput=False)
    kd = nc.declare_dram_parameter("krow", [128, KP], F32, isOutput=False)
    # dmat[k, k'] = [k==k'] - [k==k'+1]; dnext[k, k'] = -[k==0][k'==127]
    dmd = nc.declare_dram_parameter("dmat", [128, 128], F32, isOutput=False)
    dnd = nc.declare_dram_parameter("dnext", [128, 128], F32, isOutput=False)
    od = nc.declare_dram_parameter("out", [256, 256], F32, isOutput=True)

    sig = mybir.ActivationFunctionType.Sigmoid
    add = mybir.AluOpType.add

    with tile.TileContext(nc) as tc:
        with (
            tc.tile_pool(name="singles", bufs=1) as singles,
            tc.tile_pool(name="stage32", bufs=3) as stage32,
            tc.tile_pool(name="stage16", bufs=3) as stage16,
            tc.tile_pool(name="work", bufs=4) as work,
            tc.tile_pool(name="psum", bufs=1, space="PSUM") as psum,
        ):
            xt = singles.tile([128, 512], F32)
            nc.sync.dma_start(out=xt, in_=xd[:, :])
            yt = singles.tile([128, 512], F32)
            nc.sync.dma_start(out=yt, in_=yd[:, :])
            kr = singles.tile([128, KP], F32)
            nc.sync.dma_start(out=kr, in_=kd[:, :])
            dm = singles.tile([128, 128], F32)
            nc.sync.dma_start(out=dm, in_=dmd[:, :])
            dn = singles.tile([128, 128], F32)
            nc.sync.dma_start(out=dn, in_=dnd[:, :])

            # M accumulators in SBUF: rows 0..127 / 128..255 / 256 (tail)
            acc = singles.tile([128, 2, KB], F32)
            acct = singles.tile([128, KB], F32)  # only partition 0 used
            nc.vector.memset(acc, 0.0)
            nc.vector.memset(acct[0:1, :], 0.0)

            # PSUM: M' segment accumulator (rows 0..255 + tail row 256)
            Mp = psum.tile([128, 2, 512], F32)
            Mt = psum.tile([128, 512], F32)  # partition 0 = row 256

            def preadd_sigmoid(src, g, eng, tag):
                a = stage32.tile([128, XG, KP], F32, tag="a" + tag)
                tt = nc.gpsimd.tensor_tensor if eng == 'g' else \
                    nc.vector.tensor_tensor
                tt(
                    out=a,
                    in0=src[:, g * XG:(g + 1) * XG].unsqueeze(2)
                        .broadcast_to([128, XG, KP]),
                    in1=kr.unsqueeze(1).broadcast_to([128, XG, KP]),
                    op=add,
                )
                s = stage16.tile([128, XG, KP], F16, tag="s" + tag)
                nc.scalar.activation(out=s, in_=a, func=sig)
                return s

            for g in range(NG):
                sx = preadd_sigmoid(xt, g, X_ENG[g], "x")
                sy = preadd_sigmoid(yt, g, Y_ENG[g], "y")
                for i in range(XG):
                    c = g * XG + i
                    first = c % SEG == 0
                    last = c % SEG == SEG - 1
                    ty = sy[:, i, 0:KB]
                    nc.tensor.matmul(
                        Mp[:, 0, 0:KB],
                        lhsT=sx[:, i, 0:128],
                        rhs=ty,
                        start=first,
                        stop=last,
                    )
                    nc.tensor.matmul(
                        Mp[:, 1, 0:KB],
                        lhsT=sx[:, i, 128:256],
                        rhs=ty,
                        start=first,
                        stop=last,
                    )
                    nc.tensor.matmul(
                        Mt[0:1, 0:KB],
                        lhsT=sx[:, i, 256:257],
                        rhs=ty,
                        start=first,
                        stop=last,
                    )
                    if last:
                        # drain segment into SBUF accumulators
                        for h in range(2):
                            nc.vector.tensor_add(
                                out=acc[:, h, :], in0=acc[:, h, :],
                                in1=Mp[:, h, 0:KB],
                            )
                        nc.vector.tensor_add(
                            out=acct[0:1, :], in0=acct[0:1, :],
                            in1=Mt[0:1, 0:KB],
                        )

            # Epilogue: out[k, j] = (Mr[k, j] - Mr[k, j+1]) / N with
            # Mr[k, j] = acc[k, j] - acc[k+1, j], row diff via PE:
            # rd_h = dmat^T @ acc_h + dnext^T @ acc_{h+1}.
            for h in range(2):
                rd = psum.tile([128, 512], F32, tag="rd")
                nc.tensor.matmul(
                    rd[:, 0:KB], lhsT=dm, rhs=acc[:, h, :],
                    start=True, stop=False,
                )
                nxt = acc[:, 1, :] if h == 0 else acct[:, :]
                nc.tensor.matmul(
                    rd[:, 0:KB], lhsT=dn, rhs=nxt,
                    start=False, stop=True,
                )
                t1 = work.tile([128, KB], F32, tag="ep")
                nc.scalar.activation(
                    out=t1, in_=rd[:, 0:KB],
                    func=mybir.ActivationFunctionType.Copy, scale=INV_N,
                )
                t2 = work.tile([128, K], F32, tag="ep2")
                nc.vector.tensor_sub(out=t2, in0=t1[:, 0:K], in1=t1[:, 1:KB])
                nc.sync.dma_start(out=od[128 * h: 128 * (h + 1), :], in_=t2)

    nc.finalize()
    return nc


def _get_nc():
    global _cached_nc
    if _cached_nc is None:
        _cached_nc = _build()
    return _cached_nc


def _krow():
    row = np.arange(KP, dtype=np.float32) * np.float32(-2.5)
    return np.tile(row[None, :], (128, 1))


def _dmat():
    d = np.eye(128, dtype=np.float32)
    d -= np.eye(128, k=-1, dtype=np.float32)
    return d


def _dnext():
    d = np.zeros((128, 128), dtype=np.float32)
    d[0, 127] = -1.0
    return d


def _in_maps(x, y):
    x = np.asarray(x, dtype=np.float32)
    y = np.asarray(y, dtype=np.float32)
    kr = _krow()
    maps = []
    for b in range(B):
        x6 = np.ascontiguousarray(x[b].reshape(128, 512) * np.float32(640.0))
        y6 = np.ascontiguousarray(y[b].reshape(128, 512) * np.float32(640.0))
        maps.append({"x": x6, "y": y6, "krow": kr,
                     "dmat": _dmat(), "dnext": _dnext()})
    return maps


def run(x, y, trace=False, **trace_kw):
    """Run on all 8 cores; returns (out (8,256,256) f32, BassKernelResults)."""
    nc = _get_nc()
    res = run_bass_kernel_spmd(nc, _in_maps(x, y), list(range(B)), trace=trace,
                               **trace_kw)
    out = np.stack([res.results[b]["out"] for b in range(B)]).astype(np.float32)
    return out, res


def kernel(x, y):
    out, _ = run(x, y)
    return out


# revision 23
# speedup vs baseline: 1.0931x; 1.0198x over previous
"""Joint soft-histogram kernel for Trainium2 (Bass/Tile), 8-core data parallel.

Math (per batch b, K=256, L=1/256, W=L/2.5, N=65536 pixels):
    phi_k(x) = S_k(x) - S_{k+1}(x),   S_k(x) = sigmoid(640*x - 2.5*k)
    out[k, j] = sum_n phi_k(x_n) * phi_j(y_n) / N

Double telescope: out = Drow(Dcol(M)) / N with M = Sx^T @ Sy (257 x 257),
M[k, j] = sum_n S_k(x_n) * S_j(y_n). Neither side needs a per-chunk adjacent
difference -- both collapse onto the tiny M. M entries grow to O(N), so PSUM
fp32 accumulation is drained to SBUF every SEG chunks (caps entries at
SEG*128 = 8192, keeping roundoff ~1e-3 absolute, ~4e-3 relative after
differencing -- inside the 2e-2 budget).

Engine plan (v3; v1 spent 484us in per-chunk DVE TENSOR_SCALARs + 338us in
GPSIMD diffs; v2's PE rank-2 preadds measured 2.4x slower than modeled):
  - preadd A[p, c*KP+j] = 640*v[p,c] - 2.5*j: ONE broadcast-AP tensor_tensor
    per 16-chunk group (measured 4.4us/group), writing SBUF.
  - sigmoid: one big staged ACTIVATE per group (measured 3.7us/group). This
    ~238us of ACT work is the dense-algorithm floor.
  - a tunable number of preadd groups go to GPSIMD to unload DVE.
  - PE: 2x 128-row matmuls + 1-row tail matmul per chunk, fp16, plus segment
    restarts (start=True zeroes PSUM).

Sharding: pure data parallel, batch b -> core b.
"""

import numpy as np

import concourse.bass as bass
import concourse.tile as tile
from concourse import bacc, mybir
from concourse.bass_utils import run_bass_kernel_spmd

F32 = mybir.dt.float32
F16 = mybir.dt.float16

B = 8
K = 256
KB = K + 1            # 257 sigmoid taps per side (k = 0..256)
KP = K + 2            # 258: per-chunk stride in staged tiles (even)
NPIX = 65536
NCHUNK = 512
XG = 16               # chunks per staged group
NG = NCHUNK // XG     # 32 groups
GF = XG * KP          # staged group free size (4128)
INV_N = 1.0 / NPIX
SEG = 64              # chunks per PSUM accumulation segment
NSEG = NCHUNK // SEG

# --- tuning knobs -----------------------------------------------------------
# Preadd engine per (group, side): 'v' = DVE broadcast-TT, 'g' = GPSIMD TT.
X_ENG = ['v'] * NG
Y_ENG = ['g' if g % 4 == 2 else 'v' for g in range(NG)]
# ---------------------------------------------------------------------------

_cached_nc = None


def _build():
    nc = bacc.Bacc("TRN2")
    xd = nc.declare_dram_parameter("x", [128, 512], F32, isOutput=False)
    yd = nc.declare_dram_parameter("y", [128, 512], F32, isOutput=False)
    kd = nc.declare_dram_parameter("krow", [128, KP], F32, isOutput=False)
    # dmat[k, k'] = [k==k'] - [k==k'+1]; dnext[k, k'] = -[k==0][k'==127]
    dmd = nc.declare_dram_parameter("dmat", [128, 128], F32, isOutput=False)
    dnd = nc.declare_dram_parameter("dnext", [128, 128], F32, isOutput=False)
    od = nc.declare_dram_parameter("out", [256, 256], F32, isOutput=True)

    sig = mybir.ActivationFunctionType.Sigmoid
    add = mybir.AluOpType.add

    with tile.TileContext(nc) as tc:
        with (
            tc.tile_pool(name="singles", bufs=1) as singles,
            tc.tile_pool(name="stage32", bufs=3) as stage32,
            tc.tile_pool(name="stage16", bufs=3) as stage16,
            tc.tile_pool(name="work", bufs=4) as work,
            tc.tile_pool(name="psum", bufs=1, space="PSUM") as psum,
        ):
            xt = singles.tile([128, 512], F32)
            nc.sync.dma_start(out=xt, in_=xd[:, :])
            yt = singles.tile([128, 512], F32)
            nc.sync.dma_start(out=yt, in_=yd[:, :])
            kr = singles.tile([128, KP], F32)
            nc.sync.dma_start(out=kr, in_=kd[:, :])
            dm = singles.tile([128, 128], F32)
            nc.sync.dma_start(out=dm, in_=dmd[:, :])
            dn = singles.tile([128, 128], F32)
            nc.sync.dma_start(out=dn, in_=dnd[:, :])

            # M accumulators in SBUF: rows 0..127 / 128..255 / 256 (tail)
            acc = singles.tile([128, 2, KB], F32)
            acct = singles.tile([128, KB], F32)  # only partition 0 used
            nc.vector.memset(acc, 0.0)
            nc.vector.memset(acct[0:1, :], 0.0)

            # PSUM: M' segment accumulator (rows 0..255 + tail row 256)
            Mp = psum.tile([128, 2, 512], F32)
            Mt = psum.tile([128, 512], F32)  # partition 0 = row 256

            def preadd_sigmoid(src, g, eng, tag):
                a = stage32.tile([128, XG, KP], F32, tag="a" + tag)
                tt = nc.gpsimd.tensor_tensor if eng == 'g' else \
                    nc.vector.tensor_tensor
                tt(
                    out=a,
                    in0=src[:, g * XG:(g + 1) * XG].unsqueeze(2)
                        .broadcast_to([128, XG, KP]),
                    in1=kr.unsqueeze(1).broadcast_to([128, XG, KP]),
                    op=add,
                )
                s = stage16.tile([128, XG, KP], F16, tag="s" + tag)
                nc.scalar.activation(out=s, in_=a, func=sig)
                return s

            for g in range(NG):
                sx = preadd_sigmoid(xt, g, X_ENG[g], "x")
                sy = preadd_sigmoid(yt, g, Y_ENG[g], "y")
                for i in range(XG):
                    c = g * XG + i
                    first = c % SEG == 0
                    last = c % SEG == SEG - 1
                    ty = sy[:, i, 0:KB]
                    nc.tensor.matmul(
                        Mp[:, 0, 0:KB],
                        lhsT=sx[:, i, 0:128],
                        rhs=ty,
                        start=first,
                        stop=last,
                    )
                    nc.tensor.matmul(
                        Mp[:, 1, 0:KB],
                        lhsT=sx[:, i, 128:256],
                        rhs=ty,
                        start=first,
                        stop=last,
                    )
                    nc.tensor.matmul(
                        Mt[0:1, 0:KB],
                        lhsT=sx[:, i, 256:257],
                        rhs=ty,
                        start=first,
                        stop=last,
                    )
                    if last:
                        # drain segment into SBUF accumulators
                        for h in range(2):
                            nc.vector.tensor_add(
                                out=acc[:, h, :], in0=acc[:, h, :],
                                in1=Mp[:, h, 0:KB],
                            )
                        nc.vector.tensor_add(
                            out=acct[0:1, :], in0=acct[0:1, :],
                            in1=Mt[0:1, 0:KB],
                        )

            # Epilogue: out[k, j] = (Mr[k, j] - Mr[k, j+1]) / N with
            # Mr[k, j] = acc[k, j] - acc[k+1, j], row diff via PE:
            # rd_h = dmat^T @ acc_h + dnext^T @ acc_{h+1}.
            for h in range(2):
                rd = psum.tile([128, 512], F32, tag="rd")
                nc.tensor.matmul(
                    rd[:, 0:KB], lhsT=dm, rhs=acc[:, h, :],
                    start=True, stop=False,
                )
                nxt = acc[:, 1, :] if h == 0 else acct[:, :]
                nc.tensor.matmul(
                    rd[:, 0:KB], lhsT=dn, rhs=nxt,
                    start=False, stop=True,
                )
                t1 = work.tile([128, KB], F32, tag="ep")
                nc.scalar.activation(
                    out=t1, in_=rd[:, 0:KB],
                    func=mybir.ActivationFunctionType.Copy, scale=INV_N,
                )
                t2 = work.tile([128, K], F32, tag="ep2")
                nc.vector.tensor_sub(out=t2, in0=t1[:, 0:K], in1=t1[:, 1:KB])
                nc.sync.dma_start(out=od[128 * h: 128 * (h + 1), :], in_=t2)

    nc.finalize()
    return nc


def _get_nc():
    global _cached_nc
    if _cached_nc is None:
        _cached_nc = _build()
    return _cached_nc


def _krow():
    row = np.arange(KP, dtype=np.float32) * np.float32(-2.5)
    return np.tile(row[None, :], (128, 1))


def _dmat():
    d = np.eye(128, dtype=np.float32)
    d -= np.eye(128, k=-1, dtype=np.float32)
    return d


def _dnext():
    d = np.zeros((128, 128), dtype=np.float32)
    d[0, 127] = -1.0
    return d


def _in_maps(x, y):
    x = np.asarray(x, dtype=np.float32)
    y = np.asarray(y, dtype=np.float32)
    kr = _krow()
    maps = []
    for b in range(B):
        x6 = np.ascontiguousarray(x[b].reshape(128, 512) * np.float32(640.0))
        y6 = np.ascontiguousarray(y[b].reshape(128, 512) * np.float32(640.0))
        maps.append({"x": x6, "y": y6, "krow": kr,
                     "dmat": _dmat(), "dnext": _dnext()})
    return maps


def run(x, y, trace=False, **trace_kw):
    """Run on all 8 cores; returns (out (8,256,256) f32, BassKernelResults)."""
    nc = _get_nc()
    res = run_bass_kernel_spmd(nc, _in_maps(x, y), list(range(B)), trace=trace,
                               **trace_kw)
    out = np.stack([res.results[b]["out"] for b in range(B)]).astype(np.float32)
    return out, res


def kernel(x, y):
    out, _ = run(x, y)
    return out
